# revision 1
# baseline (speedup 1.0000x reference)
"""Butterfly block-sparse linear kernel for Trainium2 (8 NeuronCores, SPMD).

Computes: y = blockdiag_butterfly(x, factorL, factorR) + bias
  x:(4,2048,4096) f32, factorL/factorR:(8,512,512) f32, bias:(4096,) f32

Math (reference):
  out1[b,k,q] = sum_p x[b, 512k+p] * factorL[k,q,p]      (8 blocks of 512x512)
  z[b,l,r]    = out1_flat[b, 8r+l]                        (butterfly permute)
  out2[b,l,s] = sum_r z[b,l,r] * factorR[l,s,r]
  y[b, 8s+l]  = out2[b,l,s] + bias[8s+l]

Strategy: data-parallel over the 8192 tokens (1024 tokens/core), factors
replicated. All activations are kept feature-major on chip (features on
SBUF partitions, tokens on the free axis) so both block matmuls contract
over the partition dim. The butterfly permute becomes:
  - a host-side reordering of factorL's output channels q -> q' = 64*(q%8)+q//8
    (groups stage-1 channels by their destination stage-2 block l), and
  - an on-chip gather: each stage-1 PSUM tile (128 q' x T) splits into two
    64-partition halves (block l=2qc and l=2qc+1), which DMA (SBUF->SBUF,
    partition-remapped) into the stage-2 input tiles z[l][c].
Matmuls run as float32r (full PE rate for moving dim >= 256, ~1e-4 rel err).
Stage-2 output is evicted by ScalarE with the per-partition bias fused, then
DMA'd to HBM with row stride 8 so the final feature order j = 8s+l is already
correct; the host only transposes token-major at the end.
"""

import os
import numpy as np
from contextlib import ExitStack

NCORES = 8
TOK = 8192
TPC = TOK // NCORES          # tokens per core
TBATCH = 512                 # tokens per on-chip batch
NB = TPC // TBATCH

_CACHE = {}
LAST_RESULT = None


def _build_program():
    import concourse.bacc as bacc
    import concourse.tile as tile
    import concourse.mybir as mybir

    F32 = mybir.dt.float32
    F32R = mybir.dt.float32r

    nc = bacc.Bacc("TRN2", target_bir_lowering=False, debug=False)
    x = nc.dram_tensor("x", [4096, TPC], F32R, kind="ExternalInput").ap()
    w1 = nc.dram_tensor("w1", [128, 16384], F32R, kind="ExternalInput").ap()
    w2 = nc.dram_tensor("w2", [128, 16384], F32R, kind="ExternalInput").ap()
    bias = nc.dram_tensor("bias", [128, 32], F32, kind="ExternalInput").ap()
    out = nc.dram_tensor("out", [4096, TPC], F32, kind="ExternalOutput").ap()
    # out rows j = 1024*sc + 8*ss + l  ->  view as [sc, l, ss, t]
    out_r = out.rearrange("(a p l) t -> a l p t", p=128, l=8)

    T = TBATCH
    # x viewed per k-group: [k, pc, pp, t]
    x_r = x.rearrange("(k pc pp) t -> k pp pc t", pc=4, pp=128)

    with tile.TileContext(nc) as tc, ExitStack() as ctx:
        wpool = ctx.enter_context(tc.tile_pool(name="w", bufs=1))
        w1pool = ctx.enter_context(tc.tile_pool(name="w1p", bufs=3))
        w2pool = ctx.enter_context(tc.tile_pool(name="w2p", bufs=1))
        xpool = ctx.enter_context(tc.tile_pool(name="x", bufs=3))
        spool = ctx.enter_context(tc.tile_pool(name="stg", bufs=2))
        zpool = ctx.enter_context(tc.tile_pool(name="z", bufs=1))
        opool = ctx.enter_context(tc.tile_pool(name="o", bufs=2))
        ps1 = ctx.enter_context(tc.tile_pool(name="ps1", bufs=3, space="PSUM"))
        ps2 = ctx.enter_context(tc.tile_pool(name="ps2", bufs=3, space="PSUM"))

        bt = wpool.tile([128, 32], F32, tag="bias")
        nc.gpsimd.dma_start(bt[:], bias[:])
        # w2 stays resident all kernel; loads are paced into HBM-idle windows
        w2ts = [
            w2pool.tile([128, 2048], F32R, name=f"w2_{l}", tag=f"w2_{l}")
            for l in range(8)
        ]

        # split x/w1 per-k tiles into independent halves so the first matmuls
        # of each k-group depend on only 1MB of transfers, and emit loads in
        # an explicit software-pipeline order (cross-batch prefetch).
        loads = {}

        def emit_load(b, k):
            t0 = b * T
            qa, qb = (nc.sync, nc.scalar) if k % 2 == 0 else (nc.scalar, nc.sync)
            xta = xpool.tile([128, 2 * T], F32R, tag="xta")
            xtb = xpool.tile([128, 2 * T], F32R, tag="xtb", bufs=2)
            nc.gpsimd.dma_start(xta[:], x_r[k, :, 0:2, t0 : t0 + T])
            # batch 0's stage 1 also streams w2 on q0 -> push xtb to HW queues
            (qb if b == 0 else nc.gpsimd).dma_start(
                xtb[:], x_r[k, :, 2:4, t0 : t0 + T]
            )
            w1ta = w1pool.tile([128, 1024], F32R, tag="w1ta")
            w1tb = w1pool.tile([128, 1024], F32R, tag="w1tb")
            qb.dma_start(w1ta[:], w1[:, k * 2048 : k * 2048 + 1024])
            qa.dma_start(w1tb[:], w1[:, k * 2048 + 1024 : (k + 1) * 2048])
            loads[(b, k)] = (xta, xtb, w1ta, w1tb)

        def s1_compute(b, k):
            xta, xtb, w1ta, w1tb = loads.pop((b, k))
            xh = (xta, xtb)
            wh = (w1ta, w1tb)
            # Each qc PSUM tile splits into an aligned half (same partition
            # range as its z destination -> engine-copied directly, no DMA)
            # and a crossed half (staged, then one partition-remap DMA per k).
            # Aligned l-parity == k-parity. Even qc on DVE, odd qc on ACT so
            # the two engines never share a PSUM bank.
            c, h = k // 2, 64 * (k % 2)
            hx = 64 - h
            zv = zts[c].rearrange("p (l t) -> p l t", l=8)
            stg = spool.tile([128, 4 * T], F32R, tag="stg")
            for qc in range(4):
                p1 = ps1.tile([128, T], F32, tag="p1")
                for pc in range(4):
                    col = (pc % 2) * 512 + qc * 128
                    nc.tensor.matmul(
                        p1[:],
                        wh[pc // 2][:, col : col + 128],
                        xh[pc // 2][:, (pc % 2) * T : (pc % 2 + 1) * T],
                        start=(pc == 0),
                        stop=(pc == 3),
                    )
                l_a = 2 * qc + (k % 2)
                za = zv[h : h + 64, l_a, :]
                if qc % 2 == 0:
                    nc.vector.tensor_copy(za, p1[h : h + 64, :])
                    nc.vector.tensor_copy(
                        stg[hx : hx + 64, qc * T : (qc + 1) * T],
                        p1[hx : hx + 64, :],
                    )
                else:
                    nc.scalar.activation(
                        za, p1[h : h + 64, :],
                        mybir.ActivationFunctionType.Identity,
                    )
                    nc.scalar.activation(
                        stg[hx : hx + 64, qc * T : (qc + 1) * T],
                        p1[hx : hx + 64, :],
                        mybir.ActivationFunctionType.Identity,
                    )
            qa = nc.sync if k % 2 == 0 else nc.scalar
            qa.dma_start(
                zv[h : h + 64, (1 - k % 2) : 8 : 2, :],
                stg[hx : hx + 64, :].rearrange("p (q t) -> p q t", q=4),
            )

        def s2_compute(b, l):
            t0 = b * T
            ot = opool.tile([128, 4 * T], F32, tag="ot")
            for sc in range(4):
                p2 = ps2.tile([128, T], F32, tag="p2")
                for c in range(4):
                    col = c * 512 + sc * 128
                    nc.tensor.matmul(
                        p2[:],
                        w2ts[l][:, col : col + 128],
                        zts[c][:, l * T : (l + 1) * T],
                        start=(c == 0),
                        stop=(c == 3),
                    )
                nc.scalar.activation(
                    ot[:, sc * T : (sc + 1) * T],
                    p2[:],
                    mybir.ActivationFunctionType.Identity,
                    bias=bt[:, l * 4 + sc : l * 4 + sc + 1],
                )
            # one store per l: rows j = 1024*sc + 8*ss + l, cols t0:t0+T
            qs = nc.sync if l % 2 == 0 else nc.scalar
            qs.dma_start(
                out_r[:, l, :, t0 : t0 + T].rearrange("a p t -> p a t"),
                ot[:].rearrange("p (a t) -> p a t", a=4),
            )

        LOOK = 3
        for j in range(LOOK):
            emit_load(0, j)
        for b in range(NB):
            t0 = b * T
            # z split per r-chunk c: tile c holds [l, t] slots for r-rows
            # [128c, 128c+128); written by k=2c (parts 0:64) and k=2c+1
            zts = [
                zpool.tile([128, 8 * T], F32R, name=f"z_{c}", tag=f"z_{c}")
                for c in range(4)
            ]
            for k in range(8):
                if k + LOOK < 8:
                    emit_load(b, k + LOOK)
                if b == 0 and k >= 4:
                    # w2 l=0..3 ride the back half of batch-0 stage 1
                    nc.gpsimd.dma_start(
                        w2ts[k - 4][:], w2[:, (k - 4) * 2048 : (k - 3) * 2048]
                    )

                s1_compute(b, k)
            if b + 1 < NB:
                emit_load(b + 1, 0)
                emit_load(b + 1, 1)
            for l in range(8):
                if b + 1 < NB and l == 0:
                    emit_load(b + 1, 2)
                if b == 0 and l < 4:
                    # w2 l=4..7 two iterations ahead during batch-0 stage 2
                    nc.gpsimd.dma_start(
                        w2ts[l + 4][:], w2[:, (l + 4) * 2048 : (l + 5) * 2048]
                    )
                s2_compute(b, l)
    nc.compile()
    return nc


def _get_program():
    if "nc" not in _CACHE:
        _CACHE["nc"] = _build_program()
    return _CACHE["nc"]


def _ensure_ntff_hook():
    """Bridge the axon NTFF profile hook when the image's antenv lacks it."""
    import sys, types

    try:
        from antenv.axon_hooks import get_axon_ntff_profile_hook  # noqa: F401

        return
    except ImportError:
        pass
    try:
        from trn_agent_boot.trn_boot import _ntff_profile_via_ctypes

        hook = _ntff_profile_via_ctypes("/opt/axon/libaxon_pjrt.so")
        mod = types.ModuleType("antenv.axon_hooks")
        _h = {"hook": hook}
        mod.set_axon_ntff_profile_hook = lambda h: _h.__setitem__("hook", h)
        mod.get_axon_ntff_profile_hook = lambda: _h["hook"]
        sys.modules["antenv.axon_hooks"] = mod
        import antenv

        antenv.axon_hooks = mod
    except Exception:
        pass


def kernel(x, factorL, factorR, bias):
    global LAST_RESULT
    from concourse.bass_utils import run_bass_kernel_spmd

    x = np.asarray(x, dtype=np.float32)
    factorL = np.asarray(factorL, dtype=np.float32)
    factorR = np.asarray(factorR, dtype=np.float32)
    bias = np.asarray(bias, dtype=np.float32)

    # host-side marshalling (not device-timed)
    xt = np.ascontiguousarray(x.reshape(TOK, 4096).T)  # (4096, 8192)
    qp = np.arange(512)
    q_of_qprime = 8 * (qp % 64) + qp // 64
    w1p = factorL.transpose(0, 2, 1)[:, :, q_of_qprime]  # (8, p, q')
    w1dev = np.ascontiguousarray(
        w1p.reshape(8, 4, 128, 4, 128).transpose(2, 0, 1, 3, 4).reshape(128, 16384)
    )
    w2p = factorR.transpose(0, 2, 1)  # (8, r, s)
    w2dev = np.ascontiguousarray(
        w2p.reshape(8, 4, 128, 4, 128).transpose(2, 0, 1, 3, 4).reshape(128, 16384)
    )
    biasdev = np.ascontiguousarray(
        bias.reshape(4, 128, 8).transpose(1, 2, 0).reshape(128, 32)
    )

    in_maps = [
        {
            "x": np.ascontiguousarray(xt[:, c * TPC : (c + 1) * TPC]),
            "w1": w1dev,
            "w2": w2dev,
            "bias": biasdev,
        }
        for c in range(NCORES)
    ]
    nc = _get_program()
    trace = os.environ.get("BUTTERFLY_TRACE", "0") == "1"
    if trace:
        _ensure_ntff_hook()
    LAST_RESULT = run_bass_kernel_spmd(
        nc, in_maps, list(range(NCORES)), trace=trace
    )
    yt = np.concatenate(
        [LAST_RESULT.results[c]["out"] for c in range(NCORES)], axis=1
    )  # (4096, 8192)
    return np.ascontiguousarray(yt.T).reshape(4, 2048, 4096)



# revision 3
# speedup vs baseline: 1.8112x; 1.8112x over previous
"""Butterfly block-sparse linear kernel for Trainium2 (8 NeuronCores, SPMD).

Computes: y = blockdiag_butterfly(x, factorL, factorR) + bias
  x:(4,2048,4096) f32, factorL/factorR:(8,512,512) f32, bias:(4096,) f32

Math (reference):
  out1[b,k,q] = sum_p x[b, 512k+p] * factorL[k,q,p]      (8 blocks of 512x512)
  z[b,l,r]    = out1_flat[b, 8r+l]                        (butterfly permute)
  out2[b,l,s] = sum_r z[b,l,r] * factorR[l,s,r]
  y[b, 8s+l]  = out2[b,l,s] + bias[8s+l]

v2 strategy (vs the fp32r baseline): data-parallel over the 8192 tokens
(1024/core), everything bf16 on the wire (x, w1, w2, z, out; PSUM fp32).
This halves HBM traffic to ~25 MB/core and makes the kernel compute-bound
at the PE roofline (512 matmuls x 512 rows ~ 109 us/core). w1/w2 stay
resident in SBUF (loaded once). The butterfly permute:
  - host reorders factorL's output channels q -> q' = 64*(q%8)+q//8,
  - per (k,qc) PSUM tile: the lane-aligned 64-row half is engine-copied
    (DVE) straight into the stage-2 input tile z[c]; the crossed half is
    engine-copied (ACT) to a staging tile and one partition-remap
    SBUF->SBUF DMA per k moves all 4 qc blocks at once.
z col-order groups same-parity l blocks contiguously so the remap DMA is
fully contiguous (4KB/partition); w2/bias/out use the matching device
l-order ld (l = 2*(ld%4)+ld//4). Stage 2 runs ld=4..7 first (their deps
complete earliest) so the PE never stalls at the stage boundary. All
HBM transfers are plain 2D slices with >=4KB contiguous rows.
"""

import os
import numpy as np
from contextlib import ExitStack

NCORES = 8
TOK = 8192
TPC = TOK // NCORES          # tokens per core
T = 512                      # tokens per on-chip batch (matmul moving dim)
NB = TPC // T

_CACHE = {}
LAST_RESULT = None


def _build_program():
    import concourse.bacc as bacc
    import concourse.tile as tile
    import concourse.mybir as mybir

    F32 = mybir.dt.float32
    BF16 = mybir.dt.bfloat16
    IDENT = mybir.ActivationFunctionType.Identity

    nc = bacc.Bacc("TRN2", target_bir_lowering=False, debug=False)
    # x rows = (k, pp), cols = (b, pc, t)
    x = nc.dram_tensor("x", [1024, 4 * NB * T], BF16, kind="ExternalInput").ap()
    # w1 rows = pp, cols = (k, pc, qc, qce)
    w1 = nc.dram_tensor("w1", [128, 16384], BF16, kind="ExternalInput").ap()
    # w2 rows = p, cols = (ld, c, sc, sse)
    w2 = nc.dram_tensor("w2", [128, 16384], BF16, kind="ExternalInput").ap()
    # bias rows = ss, cols = (ld, sc)
    bias = nc.dram_tensor("bias", [128, 32], F32, kind="ExternalInput").ap()
    # out rows = ss, cols = (ld, b, sc, t)
    out = nc.dram_tensor("out", [128, 8 * NB * 4 * T], BF16,
                         kind="ExternalOutput").ap()

    with tile.TileContext(nc) as tc, ExitStack() as ctx:
        wpool = ctx.enter_context(tc.tile_pool(name="w", bufs=1))
        xpool = ctx.enter_context(tc.tile_pool(name="x", bufs=3))
        spool = ctx.enter_context(tc.tile_pool(name="stg", bufs=2))
        zpool = ctx.enter_context(tc.tile_pool(name="z", bufs=2))
        opool = ctx.enter_context(tc.tile_pool(name="o", bufs=2))
        ps1 = ctx.enter_context(tc.tile_pool(name="ps1", bufs=3, space="PSUM"))
        ps2 = ctx.enter_context(tc.tile_pool(name="ps2", bufs=3, space="PSUM"))

        bt = wpool.tile([128, 32], F32, tag="bias")
        nc.gpsimd.dma_start(bt[:], bias[:])

        # resident weights: w1 per k, w2 per ld (each [128, 2048] bf16)
        w1ts = [wpool.tile([128, 2048], BF16, name=f"w1_{k}", tag=f"w1_{k}")
                for k in range(8)]
        w2ts = [wpool.tile([128, 2048], BF16, name=f"w2_{l}", tag=f"w2_{l}")
                for l in range(8)]

        xts = {}

        def load_x(b, k, q):
            xt = xpool.tile([128, 2048], BF16, tag="xt")
            q.dma_start(xt[:], x[k * 128:(k + 1) * 128,
                               b * 2048:(b + 1) * 2048])
            xts[(b, k)] = xt

        def s1_compute(b, k):
            xt = xts.pop((b, k))
            c, h = k // 2, 64 * (k % 2)
            hx = 64 - h
            par = 1 - k % 2          # parity of the crossed l blocks
            stg = spool.tile([128, 2048], BF16, tag="stg")
            for qc in range(4):
                p1 = ps1.tile([128, T], F32, tag="p1")
                for pc in range(4):
                    nc.tensor.matmul(
                        p1[:],
                        w1ts[k][:, pc * 512 + qc * 128: pc * 512 + qc * 128 + 128],
                        xt[:, pc * T:(pc + 1) * T],
                        start=(pc == 0),
                        stop=(pc == 3),
                    )
                # aligned half: PSUM rows [h:h+64] -> z parts [h:h+64],
                # col block (par=k%2, lc=qc)
                nc.vector.tensor_copy(
                    zts[c][h:h + 64, ((k % 2) * 4 + qc) * T:
                           ((k % 2) * 4 + qc + 1) * T],
                    p1[h:h + 64, :],
                )
                # crossed half: PSUM rows [hx:hx+64] staged lane-aligned
                nc.scalar.activation(
                    stg[hx:hx + 64, qc * T:(qc + 1) * T],
                    p1[hx:hx + 64, :],
                    IDENT,
                )
            # one partition-remap DMA per k: stg parts [hx:hx+64] ->
            # z parts [h:h+64], col blocks (par, lc=0..3) contiguous
            nc.sync.dma_start(
                zts[c][h:h + 64, par * 2048:(par + 1) * 2048],
                stg[hx:hx + 64, :],
            )

        def s2_compute(b, ld):
            ot = opool.tile([128, 4 * T], BF16, tag="ot")
            for sc in range(4):
                p2 = ps2.tile([128, T], F32, tag="p2")
                for c in range(4):
                    col = c * 512 + sc * 128
                    nc.tensor.matmul(
                        p2[:],
                        w2ts[ld][:, col:col + 128],
                        zts[c][:, ld * T:(ld + 1) * T],
                        start=(c == 0),
                        stop=(c == 3),
                    )
                nc.scalar.activation(
                    ot[:, sc * T:(sc + 1) * T],
                    p2[:],
                    IDENT,
                    bias=bt[:, ld * 4 + sc:ld * 4 + sc + 1],
                )
            nc.scalar.dma_start(
                out[:, ld * (NB * 2048) + b * 2048:
                    ld * (NB * 2048) + (b + 1) * 2048],
                ot[:],
            )

        # ld order for stage 2: par=1 blocks (ld 4..7) depend on the k=7
        # aligned copies (fast engine path) and k<=6 remap DMAs, so they
        # are ready the moment stage 1 ends; par=0 (ld 0..3) wait on k=7's
        # remap DMA, which completes while ld 4..7 compute.
        LD_ORDER = [4, 5, 6, 7, 0, 1, 2, 3]

        # upfront loads: x(b0) + w1 interleaved on the two HWDGE engines
        for k in range(8):
            load_x(0, k, nc.sync if k % 2 == 0 else nc.scalar)
            (nc.scalar if k % 2 == 0 else nc.sync).dma_start(
                w1ts[k][:], w1[:, k * 2048:(k + 1) * 2048])

        for b in range(NB):
            zts = [zpool.tile([128, 8 * T], BF16, name=f"z_{b}_{c}",
                              tag=f"z_{c}") for c in range(4)]
            for k in range(8):
                if b == 0:
                    # pace w2 + next-batch x through batch 0 stage 1 (SWDGE)
                    nc.gpsimd.dma_start(
                        w2ts[k][:], w2[:, k * 2048:(k + 1) * 2048])
                    if b + 1 < NB:
                        load_x(b + 1, k, nc.gpsimd)
                s1_compute(b, k)
            for ld in LD_ORDER:
                s2_compute(b, ld)
    nc.compile()
    return nc


def _get_program():
    if "nc" not in _CACHE:
        _CACHE["nc"] = _build_program()
    return _CACHE["nc"]


def _ensure_ntff_hook():
    """Bridge the axon NTFF profile hook when the image's antenv lacks it."""
    import sys, types

    try:
        from antenv.axon_hooks import get_axon_ntff_profile_hook  # noqa: F401

        return
    except ImportError:
        pass
    try:
        from trn_agent_boot.trn_boot import _ntff_profile_via_ctypes

        hook = _ntff_profile_via_ctypes("/opt/axon/libaxon_pjrt.so")
        mod = types.ModuleType("antenv.axon_hooks")
        _h = {"hook": hook}
        mod.set_axon_ntff_profile_hook = lambda h: _h.__setitem__("hook", h)
        mod.get_axon_ntff_profile_hook = lambda: _h["hook"]
        sys.modules["antenv.axon_hooks"] = mod
        import antenv

        antenv.axon_hooks = mod
    except Exception:
        pass


def kernel(x, factorL, factorR, bias):
    global LAST_RESULT
    import ml_dtypes
    from concourse.bass_utils import run_bass_kernel_spmd

    BF16 = ml_dtypes.bfloat16
    x = np.asarray(x, dtype=np.float32)
    factorL = np.asarray(factorL, dtype=np.float32)
    factorR = np.asarray(factorR, dtype=np.float32)
    bias = np.asarray(bias, dtype=np.float32)

    # ---- host-side marshalling (not device-timed) ----
    xt = np.ascontiguousarray(x.reshape(TOK, 4096).T)  # (4096 feat, 8192 tok)

    qp = np.arange(512)
    q_of_qprime = 8 * (qp % 64) + qp // 64
    w1p = factorL.transpose(0, 2, 1)[:, :, q_of_qprime]       # (k, p, q')
    w1dev = np.ascontiguousarray(
        w1p.reshape(8, 4, 128, 4, 128).transpose(2, 0, 1, 3, 4).reshape(128, 16384)
    ).astype(BF16)

    l_of_ld = np.array([2 * (ld % 4) + ld // 4 for ld in range(8)])
    w2p = factorR.transpose(0, 2, 1)[l_of_ld]                  # (ld, r, s)
    w2dev = np.ascontiguousarray(
        w2p.reshape(8, 4, 128, 4, 128).transpose(2, 0, 1, 3, 4).reshape(128, 16384)
    ).astype(BF16)

    biasdev = np.ascontiguousarray(
        bias.reshape(4, 128, 8).transpose(1, 2, 0)[:, l_of_ld, :].reshape(128, 32)
    )

    in_maps = []
    for core in range(NCORES):
        xs = xt[:, core * TPC:(core + 1) * TPC]                # (4096, 1024)
        xd = (
            xs.reshape(8, 4, 128, NB, T)                       # k pc pp b t
            .transpose(0, 2, 3, 1, 4)                          # k pp b pc t
            .reshape(1024, 4 * NB * T)
        )
        in_maps.append({
            "x": np.ascontiguousarray(xd).astype(BF16),
            "w1": w1dev,
            "w2": w2dev,
            "bias": biasdev,
        })

    nc = _get_program()
    trace = os.environ.get("BUTTERFLY_TRACE", "0") == "1"
    if trace:
        _ensure_ntff_hook()
    LAST_RESULT = run_bass_kernel_spmd(
        nc, in_maps, list(range(NCORES)), trace=trace
    )

    # ---- unmarshal: out dev [ss, (ld, b, sc, t)] -> (4, 2048, 4096) f32 ----
    ys = []
    for core in range(NCORES):
        od = LAST_RESULT.results[core]["out"].astype(np.float32)
        od = od.reshape(128, 8, NB, 4, T)                      # ss ld b sc t
        y = od.transpose(2, 4, 3, 0, 1)                        # b t sc ss ld
        y2 = np.empty_like(y)
        y2[..., l_of_ld] = y
        ys.append(y2.reshape(TPC, 4096))
    return np.ascontiguousarray(np.concatenate(ys, axis=0)).reshape(4, 2048, 4096)
